# revision 2
# baseline (speedup 1.0000x reference)
"""Trainium2 Bass kernel for nn_DentalAnatomyLoss.

Computes, for segmentation [B=2, C=32, D=64, H=128, W=128] fp32:
  - crown/root ratio loss (per (b,c) sums over d<32 / d>=32)
  - 3D total-variation loss (mean |diff| along w, h, d)
  - returns stack([crown_root, smoothness, total_anatomy]) fp32 [3]

Strategy: pure data-parallel over the 64 (b,c) slices, 8 per NeuronCore.
Each core reduces its 32 MiB shard to a [128, 96] fp32 partial tensor;
the host combines partials into the 3 scalars.

Per-core engine split (memory-regime, ~94 us HBM roofline/core):
  - ScalarE: fp32->bf16 cast with fused accum (crown/root sums), and
    Abs+accum consuming the h-diff matmul output from PSUM.
  - VectorE: w-diff and d-diff subtracts (bf16, aligned 2x mode) and
    abs+sum via tensor_scalar(abs_max, accum_out).
  - TensorE: bidiagonal matmul computes h-diffs (partition axis) in PSUM.
  - DMA: HBM loads + an sbuf->sbuf shifted copy that makes the w-diff
    operands 4B-aligned (2x mode) instead of a 1x misaligned subtract.
"""

import os

import numpy as np

B, C, D, H, W = 2, 32, 64, 128, 128
NCORES = 8
JPC = (B * C) // NCORES  # (b,c) slices per core
CROWN_ROOT_W = 2.0
SMOOTH_W = 1.5
EXPECTED_RATIO = 1.2

# accumulator column layout in the [128, ACC_COLS] partial tensor.
# |d| sums are split as sum(max(d,0)) and sum(min(d,0)) because walrus
# rejects abs_max in the fused tensor_scalar+reduce form; host takes P-N.
ACC_COLS = 128
COL_X = 0  # 16: sum(x) per chunk (j*2+half)
COL_DXP = 16  # 16: sum(max(dx,0)) per chunk
COL_DXN = 32  # 16: sum(min(dx,0)) per chunk
COL_DZP = 48  # 16: sum(max(dz,0)) per chunk (in-chunk pairs)
COL_DZN = 64  # 16: sum(min(dz,0))
COL_DY = 80  # 16: sum|dy| per chunk
COL_DZBP = 96  # 8: boundary-pair sum(max) per slice j
COL_DZBN = 104  # 8: boundary-pair sum(min)
# 112:128 unused (zeroed)

_PROG_CACHE: dict = {}
last_exec_time_ns = None  # set by kernel() when tracing is enabled


def _build_program(jpc=JPC, d=D, h=H, w=W, use_xbs_dma=True):
    """Build the (single) SPMD Bass program run identically on all cores."""
    from contextlib import ExitStack

    import concourse.bass as bass  # noqa: F401
    import concourse.tile as tile
    from concourse import bacc, mybir

    f32 = mybir.dt.float32
    bf16 = mybir.dt.bfloat16
    AO = mybir.AluOpType
    AF = mybir.ActivationFunctionType

    ndh = d // 2  # planes per chunk; chunks never straddle the crown/root split
    fsz = ndh * w  # free size of one chunk
    nchunk = jpc * 2

    nc = bacc.Bacc(
        "TRN2",
        target_bir_lowering=False,
        debug=False,
        enable_asserts=False,
        num_devices=NCORES,
    )
    seg = nc.dram_tensor("seg", [jpc, d, h, w], f32, kind="ExternalInput").ap()
    bd = nc.dram_tensor("bidiag", [h, h], bf16, kind="ExternalInput").ap()
    out = nc.dram_tensor("partials", [h, ACC_COLS], f32, kind="ExternalOutput").ap()

    with tile.TileContext(nc) as tc, ExitStack() as ctx:
        singles = ctx.enter_context(tc.tile_pool(name="singles", bufs=1))
        x32p = ctx.enter_context(tc.tile_pool(name="x32", bufs=2))
        xbp = ctx.enter_context(tc.tile_pool(name="xb", bufs=3))
        xbsp = ctx.enter_context(tc.tile_pool(name="xbs", bufs=2))
        dxp = ctx.enter_context(tc.tile_pool(name="dx", bufs=2))
        dzp = ctx.enter_context(tc.tile_pool(name="dz", bufs=2))
        tinyp = ctx.enter_context(tc.tile_pool(name="tiny", bufs=2))
        dummyp = ctx.enter_context(tc.tile_pool(name="dummy", bufs=4))
        psp = ctx.enter_context(tc.tile_pool(name="ps", bufs=1, space="PSUM"))

        bd_sb = singles.tile([h, h], bf16)
        nc.sync.dma_start(out=bd_sb, in_=bd)
        acc = singles.tile([h, ACC_COLS], f32)
        nc.vector.memset(acc, 0.0)

        nblk = fsz // 512  # matmul free-dim blocks (512 = one PSUM bank)

        def emit_absum(src_ap, n, pcol, ncol):
            """acc[:,pcol] = sum(max(src,0)); acc[:,ncol] = sum(min(src,0))."""
            for op, col in ((AO.max, pcol), (AO.min, ncol)):
                dmy = dummyp.tile([h, 1], bf16)
                nc.vector.tensor_scalar(
                    out=dmy.broadcast_to((h, n)),
                    in0=src_ap,
                    scalar1=0.0,
                    scalar2=None,
                    op0=op,
                    op1=AO.add,
                    accum_out=acc[:, col : col + 1],
                )


        prev_xb = None
        pending_gy = None  # deferred ScalarE Abs+accum: (psum_tile, chunk_idx)

        def emit_gy(ps_tile, cidx):
            dya = dummyp.tile([h, 1], bf16)
            nc.scalar.activation(
                out=dya.broadcast_to((h, nblk, 512)),
                in_=ps_tile[:, :, :],
                func=AF.Abs,
                accum_out=acc[:, COL_DY + cidx : COL_DY + cidx + 1],
            )

        for j in range(jpc):
            for half in range(2):
                cidx = j * 2 + half
                d0 = half * ndh

                # 1) load chunk: [h partitions, ndh planes, w] fp32
                x32 = x32p.tile([h, ndh, w], f32)
                nc.sync.dma_start(
                    out=x32, in_=seg[j, d0 : d0 + ndh, :, :].rearrange("d h w -> h d w")
                )

                # 2) cast to bf16; fused accum -> crown/root sum for this chunk
                xb = xbp.tile([h, ndh, w], bf16)
                nc.scalar.activation(
                    out=xb,
                    in_=x32,
                    func=AF.Copy,
                    accum_out=acc[:, COL_X + cidx : COL_X + cidx + 1],
                )
                xbf = xb.rearrange("p a b -> p (a b)")

                # 3) w-diff (gx)
                dx = dxp.tile([h, fsz], bf16)
                if use_xbs_dma:
                    # shifted copy: xbs[f] = xb[f+1]; then overwrite each
                    # plane's last column with xb's so the wrapped pairs
                    # difference to exactly zero.
                    xbs = xbsp.tile([h, fsz], bf16)
                    nc.scalar.dma_start(out=xbs[:, 0 : fsz - 1], in_=xbf[:, 1:fsz])
                    nc.vector.tensor_copy(
                        xbs.rearrange("p (a b) -> p a b", b=w)[:, :, w - 1 :],
                        xb[:, :, w - 1 :],
                    )
                    nc.vector.tensor_tensor(out=dx, in0=xbf, in1=xbs, op=AO.subtract)
                else:
                    # direct misaligned 3D-AP subtract (1x mode, no junk pairs)
                    nc.vector.tensor_tensor(
                        out=dx.rearrange("p (a b) -> p a b", b=w)[:, :, 0 : w - 1],
                        in0=xb[:, :, 1:],
                        in1=xb[:, :, 0 : w - 1],
                        op=AO.subtract,
                    )
                    nc.vector.memset(
                        dx.rearrange("p (a b) -> p a b", b=w)[:, :, w - 1 :], 0.0
                    )
                emit_absum(dx, fsz, COL_DXP + cidx, COL_DXN + cidx)

                # 4) d-diff (gz), in-chunk pairs: planes 1..ndh-1 minus 0..ndh-2
                dz = dzp.tile([h, fsz - w], bf16)
                nc.vector.tensor_tensor(
                    out=dz, in0=xbf[:, w:fsz], in1=xbf[:, 0 : fsz - w], op=AO.subtract
                )
                emit_absum(dz, fsz - w, COL_DZP + cidx, COL_DZN + cidx)

                # 5) d-diff boundary pair between the two halves of slice j
                if half == 1:
                    bnd = tinyp.tile([h, w], bf16)
                    nc.vector.tensor_tensor(
                        out=bnd,
                        in0=xb[:, 0, :],
                        in1=prev_xb[:, ndh - 1, :],
                        op=AO.subtract,
                    )
                    emit_absum(bnd, w, COL_DZBP + j, COL_DZBN + j)
                prev_xb = xb

                # 6) h-diff (gy) via bidiagonal matmul into PSUM
                ps = psp.tile([h, nblk, 512], f32)
                planes_per_blk = 512 // w
                for blk in range(nblk):
                    nc.tensor.matmul(
                        ps[:, blk, :],
                        bd_sb,
                        xb[:, blk * planes_per_blk : (blk + 1) * planes_per_blk, :],
                        start=True,
                        stop=True,
                    )
                # 7) consume previous chunk's PSUM (software pipelining: keeps
                #    ScalarE busy with the next cast while PE fills this PSUM)
                if pending_gy is not None:
                    emit_gy(*pending_gy)
                pending_gy = (ps, cidx)

        emit_gy(*pending_gy)
        nc.sync.dma_start(out=out, in_=acc)

    nc.compile()
    return nc


def _get_program():
    key = "full"
    if key not in _PROG_CACHE:
        _PROG_CACHE[key] = _build_program()
    return _PROG_CACHE[key]


def _bidiag_np(h=H):
    import ml_dtypes

    m = np.zeros((h, h), dtype=np.float32)
    for c in range(h - 1):
        m[c + 1, c] = 1.0
        m[c, c] = -1.0
    # last column stays zero -> output row h-1 is 0
    return m.astype(ml_dtypes.bfloat16)


def _combine(partials, b=B, c=C, d=D, h=H, w=W):
    """Host-side finish: 8 x [128, 96] fp32 partials -> [3] fp32."""
    nslice = b * c
    jpc = nslice // len(partials)
    ndh = d // 2

    crown = np.zeros(nslice, dtype=np.float64)
    root = np.zeros(nslice, dtype=np.float64)
    gx_sum = 0.0
    gy_sum = 0.0
    gz_sum = 0.0
    for k, p in enumerate(partials):
        p = p.astype(np.float64)
        for jj in range(jpc):
            crown[k * jpc + jj] = p[:, COL_X + 2 * jj].sum()
            root[k * jpc + jj] = p[:, COL_X + 2 * jj + 1].sum()
        gx_sum += (
            p[:, COL_DXP : COL_DXP + 2 * jpc].sum()
            - p[:, COL_DXN : COL_DXN + 2 * jpc].sum()
        )
        gy_sum += p[:, COL_DY : COL_DY + 2 * jpc].sum()
        gz_sum += (
            p[:, COL_DZP : COL_DZP + 2 * jpc].sum()
            - p[:, COL_DZN : COL_DZN + 2 * jpc].sum()
            + p[:, COL_DZBP : COL_DZBP + jpc].sum()
            - p[:, COL_DZBN : COL_DZBN + jpc].sum()
        )

    total = crown + root
    valid = (total > 0) & (root > 0)
    safe_root = np.where(root > 0, root, 1.0)
    ratio_loss = np.where(valid, (crown / safe_root - EXPECTED_RATIO) ** 2, 0.0)
    cr_loss = ratio_loss.sum() / nslice

    nx = nslice * d * h * (w - 1)
    ny = nslice * d * (h - 1) * w
    nz = nslice * (d - 1) * h * w
    tv = gx_sum / nx + gy_sum / ny + gz_sum / nz

    crown_root = cr_loss * CROWN_ROOT_W
    smoothness = tv * SMOOTH_W
    return np.array(
        [crown_root, smoothness, crown_root + smoothness], dtype=np.float32
    )


def kernel(segmentation: np.ndarray) -> np.ndarray:
    global last_exec_time_ns
    from concourse.bass_utils import run_bass_kernel_spmd

    seg = np.ascontiguousarray(np.asarray(segmentation), dtype=np.float32)
    assert seg.shape == (B, C, D, H, W)
    nc = _get_program()

    bd = _bidiag_np()
    shards = seg.reshape(B * C, D, H, W)
    in_maps = [
        {"seg": np.ascontiguousarray(shards[k * JPC : (k + 1) * JPC]), "bidiag": bd}
        for k in range(NCORES)
    ]
    trace = bool(os.environ.get("BASS_TRACE"))
    res = run_bass_kernel_spmd(nc, in_maps, list(range(NCORES)), trace=trace)
    last_exec_time_ns = res.exec_time_ns
    partials = [res.results[k]["partials"] for k in range(NCORES)]
    return _combine(partials)


# revision 11
# speedup vs baseline: 17642.5231x; 17642.5231x over previous
"""Trainium2 Bass kernel for nn_DentalAnatomyLoss.

Computes, for segmentation [B=2, C=32, D=64, H=128, W=128] fp32:
  - crown/root ratio loss (per (b,c) sums over d<32 / d>=32)
  - 3D total-variation loss (mean |diff| along w, h, d)
  - returns stack([crown_root, smoothness, total_anatomy]) fp32 [3]

Strategy: pure data-parallel over the 64 (b,c) slices, 8 per NeuronCore.
Each core reduces its 32 MiB shard to a [128, 160] fp32 partial tensor;
the host combines partials into the 3 scalars.

Per-core engine split (memory regime, ~94 us HBM roofline/core):
  - ScalarE: fp32->bf16 cast with fused accum_out (crown/root sums), and
    Abs+accum_out consuming the h-diff matmul output from PSUM.
  - VectorE: one fused scalar_tensor_tensor per diff direction computes
    out=max(a,b) with accum_out=sum(max(a,b)); the host recovers
    sum|a-b| = 2*sum(max(a,b)) - sum(a) - sum(b), where sum(a)/sum(b)
    come from the cast's fused accum and tiny boundary-column sums.
  - TensorE: bidiagonal matmul computes h-diffs (partition axis) in PSUM.
  - DMA: HBM loads only (the SP ring), ~94 us/core at ~360 GB/s.
"""

import os

import numpy as np

B, C, D, H, W = 2, 32, 64, 128, 128
NCORES = 8
JPC = (B * C) // NCORES  # (b,c) slices per core
CROWN_ROOT_W = 2.0
SMOOTH_W = 1.5
EXPECTED_RATIO = 1.2

# accumulator column layout in the [128, ACC_COLS] partial tensor
# (one column per chunk = (slice j, half); 16 chunks per core)
ACC_COLS = 160
COL_X = 0  # 16: sum(x) per chunk
COL_DXP = 16  # 16: sum(max(x[...,w], x[...,w+1])) over w-pairs
COL_TXF = 32  # 16: sum over planes of column w=0
COL_TXL = 48  # 16: sum over planes of column w=W-1
COL_DZP = 64  # 16: sum(max(dz,0)), dz = plane[k+1]-plane[k] (in-chunk)
COL_TZF = 80  # 16: sum of first plane of chunk
COL_TZL = 96  # 16: sum of last plane of chunk
COL_DY = 112  # 16: sum|dy| (ScalarE Abs accum from PSUM)
COL_BNDP = 128  # 8: sum(max(a,b)) for the half0/half1 boundary plane pair
# 136:160 unused (zeroed)

_PROG_CACHE: dict = {}
last_exec_time_ns = None  # set by kernel() when tracing is enabled


def _build_program(jpc=JPC, d=D, h=H, w=W, repeat=1, skip=()):
    """Build the (single) SPMD Bass program run identically on all cores.

    repeat>1 wraps the whole compute in a hardware For_i loop (identical
    result, used only for wall-clock timing of the kernel body).
    """
    from contextlib import ExitStack

    import concourse.tile as tile
    from concourse import bacc, mybir

    f32 = mybir.dt.float32
    bf16 = mybir.dt.bfloat16
    AO = mybir.AluOpType
    AF = mybir.ActivationFunctionType

    ndh = d // 2  # planes per chunk; chunks never straddle the crown/root split
    fsz = ndh * w  # free size of one chunk

    nc = bacc.Bacc(
        "TRN2",
        target_bir_lowering=False,
        debug=False,
        enable_asserts=False,
        num_devices=NCORES,
    )
    seg = nc.dram_tensor("seg", [jpc, d, h, w], f32, kind="ExternalInput").ap()
    bd = nc.dram_tensor("bidiag", [h, h], bf16, kind="ExternalInput").ap()
    out = nc.dram_tensor("partials", [h, ACC_COLS], f32, kind="ExternalOutput").ap()

    with tile.TileContext(nc) as tc, ExitStack() as ctx:
        singles = ctx.enter_context(tc.tile_pool(name="singles", bufs=1))
        x32p = ctx.enter_context(tc.tile_pool(name="x32", bufs=3))
        xbp = ctx.enter_context(tc.tile_pool(name="xb", bufs=3))
        dxp = ctx.enter_context(tc.tile_pool(name="dx", bufs=2))
        dzp = ctx.enter_context(tc.tile_pool(name="dz", bufs=2))
        tinyp = ctx.enter_context(tc.tile_pool(name="tiny", bufs=2))
        dummyp = ctx.enter_context(tc.tile_pool(name="dummy", bufs=4))
        psp = ctx.enter_context(tc.tile_pool(name="ps", bufs=1, space="PSUM"))

        bd_sb = singles.tile([h, h], bf16)
        nc.sync.dma_start(out=bd_sb, in_=bd)
        acc = singles.tile([h, ACC_COLS], f32)
        nc.vector.memset(acc, 0.0)

        nblk = fsz // 512  # matmul free-dim blocks (512 = one PSUM bank)
        planes_per_blk = 512 // w

        def sum_max(out_ap, a_ap, b_ap, col):
            """out = max(a,b); acc[:,col] = sum(out). out is write-only."""
            nc.vector.scalar_tensor_tensor(
                out=out_ap,
                in0=a_ap,
                scalar=0.0,
                in1=b_ap,
                op0=AO.bypass,
                op1=AO.max,
                accum_out=acc[:, col : col + 1],
            )

        def sum_relu(src_ap, col):
            """acc[:,col] = sum(max(src,0)); src rewritten in place."""
            nc.vector.tensor_scalar(
                out=src_ap,
                in0=src_ap,
                scalar1=0.0,
                scalar2=None,
                op0=AO.max,
                op1=AO.add,
                accum_out=acc[:, col : col + 1],
            )

        def sum_ident(src_ap, col):
            """acc[:,col] = sum(src); src rewritten in place (x + 0.0).

            Only used on tiles of non-negative values (x in [0,1)), so the
            identity rewrite is bit-exact.
            """
            nc.vector.tensor_scalar(
                out=src_ap,
                in0=src_ap,
                scalar1=0.0,
                scalar2=None,
                op0=AO.add,
                op1=AO.add,
                accum_out=acc[:, col : col + 1],
            )

        state = {"prev_xb": None, "pending_gy": None}

        def emit_gy(ps_tile, cidx):
            dya = dummyp.tile([h, 1], bf16)
            nc.scalar.activation(
                out=dya.broadcast_to((h, nblk, 512)),
                in_=ps_tile[:, :, :],
                func=AF.Abs,
                accum_out=acc[:, COL_DY + cidx : COL_DY + cidx + 1],
            )

        def chunk_body(j, half):
                cidx = j * 2 + half
                d0 = half * ndh

                # 1) load chunk: [h partitions, ndh planes, w] fp32
                x32 = x32p.tile([h, ndh, w], f32)
                nc.sync.dma_start(
                    out=x32, in_=seg[j, d0 : d0 + ndh, :, :].rearrange("d h w -> h d w")
                )

                # 2) cast to bf16; fused accum -> crown/root sum for this chunk
                if "conv" in skip:
                    return
                xb = xbp.tile([h, ndh, w], bf16)
                nc.scalar.activation(
                    out=xb,
                    in_=x32,
                    func=AF.Copy,
                    accum_out=acc[:, COL_X + cidx : COL_X + cidx + 1],
                )
                xbf = xb.rearrange("p a b -> p (a b)")

                # 3) w-diff (gx): one fused op per chunk.  The exact 3D AP
                #    (misaligned by one element) runs at 1x either way, so it
                #    reads the fp32 tile directly: no dependency on the cast,
                #    and full fp32 precision for the gx term.
                if "dx" not in skip:
                    dx = dxp.tile([h, ndh, w - 1], bf16)
                    sum_max(dx, x32[:, :, 1:], x32[:, :, 0 : w - 1], COL_DXP + cidx)
                    # boundary-column sums for the signed sums (fp32)
                    sum_ident(x32[:, :, 0:1], COL_TXF + cidx)
                    sum_ident(x32[:, :, w - 1 : w], COL_TXL + cidx)

                # 4) d-diff (gz), in-chunk pairs: aligned TT subtract (2x)
                #    then fused relu-sum (4x); sum(dz) telescopes on host.
                if "dz" not in skip:
                    dz = dzp.tile([h, fsz - w], bf16)
                    nc.vector.tensor_tensor(
                        out=dz,
                        in0=xbf[:, w:fsz],
                        in1=xbf[:, 0 : fsz - w],
                        op=AO.subtract,
                    )
                    sum_relu(dz[:, :], COL_DZP + cidx)
                    # first/last plane sums for the signed sums
                    sum_ident(xb[:, 0, :], COL_TZF + cidx)
                    sum_ident(xb[:, ndh - 1, :], COL_TZL + cidx)

                    # 5) boundary pair between the two halves of slice j
                    if half == 1:
                        bnd = tinyp.tile([h, w], bf16)
                        sum_max(
                            bnd,
                            xb[:, 0, :],
                            state["prev_xb"][:, ndh - 1, :],
                            COL_BNDP + j,
                        )
                    state["prev_xb"] = xb

                # 6) h-diff (gy) via bidiagonal matmul into PSUM (all 8 banks)
                if "gy" in skip:
                    return
                ps = psp.tile([h, nblk, 512], f32)
                for blk in range(nblk):
                    nc.tensor.matmul(
                        ps[:, blk, :],
                        bd_sb,
                        xb[:, blk * planes_per_blk : (blk + 1) * planes_per_blk, :],
                        start=True,
                        stop=True,
                    )
                # 7) consume previous chunk's PSUM (software pipelining: keeps
                #    ScalarE busy with the next cast while PE fills this PSUM)
                if state["pending_gy"] is not None:
                    emit_gy(*state["pending_gy"])
                state["pending_gy"] = (ps, cidx)

        def all_chunks():
            for j in range(jpc):
                for half in range(2):
                    chunk_body(j, half)
            if state["pending_gy"] is not None:
                emit_gy(*state["pending_gy"])
            state["pending_gy"] = None

        if repeat == 1:
            all_chunks()
        else:
            with tc.For_i(0, repeat, 1):
                all_chunks()
        nc.sync.dma_start(out=out, in_=acc)

    nc.compile()
    return nc


def _get_program():
    key = "full"
    if key not in _PROG_CACHE:
        _PROG_CACHE[key] = _build_program()
    return _PROG_CACHE[key]


def _bidiag_np(h=H):
    """lhsT for the h-diff matmul: out[m,:] = rhs[m+1,:] - rhs[m,:]."""
    import ml_dtypes

    m = np.zeros((h, h), dtype=np.float32)
    for c in range(h - 1):
        m[c + 1, c] = 1.0
        m[c, c] = -1.0
    # last column stays zero -> output row h-1 is 0
    return m.astype(ml_dtypes.bfloat16)


def _combine(partials, b=B, c=C, d=D, h=H, w=W):
    """Host-side finish: per-core [128, 160] fp32 partials -> [3] fp32."""
    nslice = b * c
    jpc = nslice // len(partials)

    crown = np.zeros(nslice, dtype=np.float64)
    root = np.zeros(nslice, dtype=np.float64)
    gx_sum = 0.0
    gy_sum = 0.0
    gz_sum = 0.0
    for k, p in enumerate(partials):
        p = p.astype(np.float64)
        xp = p[:, COL_DXP : COL_DXP + 2 * jpc].sum(axis=0)
        txf = p[:, COL_TXF : COL_TXF + 2 * jpc].sum(axis=0)
        txl = p[:, COL_TXL : COL_TXL + 2 * jpc].sum(axis=0)
        zp = p[:, COL_DZP : COL_DZP + 2 * jpc].sum(axis=0)
        tzf = p[:, COL_TZF : COL_TZF + 2 * jpc].sum(axis=0)
        tzl = p[:, COL_TZL : COL_TZL + 2 * jpc].sum(axis=0)
        bndp = p[:, COL_BNDP : COL_BNDP + jpc].sum(axis=0)

        xs = p[:, COL_X : COL_X + 2 * jpc].sum(axis=0)
        # sum|a-b| = 2*sum(max(a,b)) - sum(a) - sum(b)
        # gx: a = x[..., 1:], b = x[..., :-1]
        gx_sum += (2.0 * xp - (xs - txf) - (xs - txl)).sum()
        # gz: dz = planes[1:] - planes[:-1]; sum(dz) = tzl - tzf
        gz_sum += (2.0 * zp - (tzl - tzf)).sum()
        # boundary pair: a = half1.plane0, b = half0.plane(ndh-1)
        for jj in range(jpc):
            gz_sum += 2.0 * bndp[jj] - tzf[2 * jj + 1] - tzl[2 * jj]
        gy_sum += p[:, COL_DY : COL_DY + 2 * jpc].sum()

        for jj in range(jpc):
            crown[k * jpc + jj] = p[:, COL_X + 2 * jj].sum()
            root[k * jpc + jj] = p[:, COL_X + 2 * jj + 1].sum()

    total = crown + root
    valid = (total > 0) & (root > 0)
    safe_root = np.where(root > 0, root, 1.0)
    ratio_loss = np.where(valid, (crown / safe_root - EXPECTED_RATIO) ** 2, 0.0)
    cr_loss = ratio_loss.sum() / nslice

    nx = nslice * d * h * (w - 1)
    ny = nslice * d * (h - 1) * w
    nz = nslice * (d - 1) * h * w
    tv = gx_sum / nx + gy_sum / ny + gz_sum / nz

    crown_root = cr_loss * CROWN_ROOT_W
    smoothness = tv * SMOOTH_W
    return np.array(
        [crown_root, smoothness, crown_root + smoothness], dtype=np.float32
    )


def kernel(segmentation: np.ndarray) -> np.ndarray:
    global last_exec_time_ns
    from concourse.bass_utils import run_bass_kernel_spmd

    seg = np.ascontiguousarray(np.asarray(segmentation), dtype=np.float32)
    assert seg.shape == (B, C, D, H, W)
    nc = _get_program()

    bd = _bidiag_np()
    shards = seg.reshape(B * C, D, H, W)
    in_maps = [
        {"seg": np.ascontiguousarray(shards[k * JPC : (k + 1) * JPC]), "bidiag": bd}
        for k in range(NCORES)
    ]
    trace = bool(os.environ.get("BASS_TRACE"))
    res = run_bass_kernel_spmd(nc, in_maps, list(range(NCORES)), trace=trace)
    last_exec_time_ns = res.exec_time_ns
    partials = [res.results[k]["partials"] for k in range(NCORES)]
    return _combine(partials)


# revision 12
# speedup vs baseline: 18189.6850x; 1.0310x over previous
"""Trainium2 Bass kernel for nn_DentalAnatomyLoss.

Computes, for segmentation [B=2, C=32, D=64, H=128, W=128] fp32:
  - crown/root ratio loss (per (b,c) sums over d<32 / d>=32)
  - 3D total-variation loss (mean |diff| along w, h, d)
  - returns stack([crown_root, smoothness, total_anatomy]) fp32 [3]

Strategy: pure data-parallel over the 64 (b,c) slices, 8 per NeuronCore.
Each core reduces its 32 MiB shard to a [128, 160] fp32 partial tensor;
the host combines partials into the 3 scalars.

Per-core engine split (memory regime, ~94 us HBM roofline/core):
  - ScalarE: fp32->bf16 cast with fused accum_out (crown/root sums), and
    Abs+accum_out consuming the h-diff matmul output from PSUM.
  - VectorE: the w-diff as one fused scalar_tensor_tensor (out=max(a,b),
    accum_out=sum) reading fp32 directly (the shift-by-one AP is 1x in
    any dtype); the d-diff as an aligned 2x subtract + 4x fused relu-sum.
    The host recovers sum|a-b| = 2*sum(max(a,b)) - sum(a) - sum(b) and
    sum|d| = 2*sum(max(d,0)) - sum(d), with the signed sums telescoping
    to boundary-column sums.
  - TensorE: bidiagonal matmul computes h-diffs (partition axis) in PSUM.
  - DMA: HBM loads only (the SP ring), ~94 us/core at ~360 GB/s.

Pipelining: xb-dependent work (d-diff, h-diff matmul) is emitted one
chunk late so VectorE never waits on the cast; PSUM is two half-chunk
tiles (4 banks each) so TensorE fills one while ScalarE drains the
other; each PSUM drain is deferred past the next fill.
"""

import os

import numpy as np

B, C, D, H, W = 2, 32, 64, 128, 128
NCORES = 8
JPC = (B * C) // NCORES  # (b,c) slices per core
CROWN_ROOT_W = 2.0
SMOOTH_W = 1.5
EXPECTED_RATIO = 1.2

# accumulator column layout in the [128, ACC_COLS] partial tensor
# (one column per chunk = (slice j, half); 16 chunks per core)
ACC_COLS = 160
COL_X = 0  # 16: sum(x) per chunk
COL_DXP = 16  # 16: sum(max(x[...,w], x[...,w+1])) over w-pairs
COL_TXF = 32  # 16: sum over planes of column w=0
COL_TXL = 48  # 16: sum over planes of column w=W-1
COL_DZP = 64  # 16: sum(max(dz,0)), dz = plane[k+1]-plane[k] (in-chunk)
COL_TZF = 80  # 16: sum of first plane of chunk
COL_TZL = 96  # 16: sum of last plane of chunk
COL_DY = 112  # 32: sum|dy| per (chunk, psum-half)
COL_BNDP = 144  # 8: sum(max(a,b)) for the half0/half1 boundary plane pair
# 152:160 unused (zeroed)

_PROG_CACHE: dict = {}
last_exec_time_ns = None  # set by kernel() when tracing is enabled


def _build_program(jpc=JPC, d=D, h=H, w=W, repeat=1, skip=()):
    """Build the (single) SPMD Bass program run identically on all cores.

    repeat>1 wraps the whole compute in a hardware For_i loop (identical
    result, used only for wall-clock timing of the kernel body).
    """
    from contextlib import ExitStack

    import concourse.tile as tile
    from concourse import bacc, mybir

    f32 = mybir.dt.float32
    bf16 = mybir.dt.bfloat16
    AO = mybir.AluOpType
    AF = mybir.ActivationFunctionType

    ndh = d // 2  # planes per chunk; chunks never straddle the crown/root split
    fsz = ndh * w  # free size of one chunk

    nc = bacc.Bacc(
        "TRN2",
        target_bir_lowering=False,
        debug=False,
        enable_asserts=False,
        num_devices=NCORES,
    )
    seg = nc.dram_tensor("seg", [jpc, d, h, w], f32, kind="ExternalInput").ap()
    bd = nc.dram_tensor("bidiag", [h, h], bf16, kind="ExternalInput").ap()
    out = nc.dram_tensor("partials", [h, ACC_COLS], f32, kind="ExternalOutput").ap()

    with tile.TileContext(nc) as tc, ExitStack() as ctx:
        singles = ctx.enter_context(tc.tile_pool(name="singles", bufs=1))
        x32p = ctx.enter_context(tc.tile_pool(name="x32", bufs=3))
        xbp = ctx.enter_context(tc.tile_pool(name="xb", bufs=4))
        dxp = ctx.enter_context(tc.tile_pool(name="dx", bufs=2))
        dzp = ctx.enter_context(tc.tile_pool(name="dz", bufs=2))
        tinyp = ctx.enter_context(tc.tile_pool(name="tiny", bufs=2))
        dummyp = ctx.enter_context(tc.tile_pool(name="dummy", bufs=4))
        psp = ctx.enter_context(tc.tile_pool(name="ps", bufs=2, space="PSUM"))

        bd_sb = singles.tile([h, h], bf16)
        nc.sync.dma_start(out=bd_sb, in_=bd)
        acc = singles.tile([h, ACC_COLS], f32)
        nc.vector.memset(acc, 0.0)

        nblk = fsz // 512  # matmul free-dim blocks (512 = one PSUM bank)
        planes_per_blk = 512 // w
        nsub = 2 if nblk % 2 == 0 and nblk >= 2 else 1
        hb = nblk // nsub  # psum blocks per half-chunk tile

        def sum_max(out_ap, a_ap, b_ap, col):
            """out = max(a,b); acc[:,col] = sum(out). out is write-only."""
            nc.vector.scalar_tensor_tensor(
                out=out_ap,
                in0=a_ap,
                scalar=0.0,
                in1=b_ap,
                op0=AO.bypass,
                op1=AO.max,
                accum_out=acc[:, col : col + 1],
            )

        def sum_relu(src_ap, col):
            """acc[:,col] = sum(max(src,0)); src rewritten in place."""
            nc.vector.tensor_scalar(
                out=src_ap,
                in0=src_ap,
                scalar1=0.0,
                scalar2=None,
                op0=AO.max,
                op1=AO.add,
                accum_out=acc[:, col : col + 1],
            )

        def sum_ident(src_ap, col):
            """acc[:,col] = sum(src); src rewritten in place (x + 0.0).

            Only used on tiles of non-negative values (x in [0,1)), so the
            identity rewrite is bit-exact.
            """
            nc.vector.tensor_scalar(
                out=src_ap,
                in0=src_ap,
                scalar1=0.0,
                scalar2=None,
                op0=AO.add,
                op1=AO.add,
                accum_out=acc[:, col : col + 1],
            )

        state = {"prev_xb": None, "pending_gy": None, "pending_c": None}

        def emit_gy(ps_tile, cidx, sub):
            dya = dummyp.tile([h, 1], bf16)
            col = COL_DY + nsub * cidx + sub
            nc.scalar.activation(
                out=dya.broadcast_to((h, hb, 512)),
                in_=ps_tile[:, :, :],
                func=AF.Abs,
                accum_out=acc[:, col : col + 1],
            )

        def stage_c(j, half, cidx, xb, xbf):
            """xb-dependent work, emitted one chunk late (see module doc)."""
            # d-diff (gz), in-chunk pairs: aligned TT subtract (2x) then
            # fused relu-sum (4x); sum(dz) telescopes on host.
            if "dz" not in skip:
                dz = dzp.tile([h, fsz - w], bf16)
                nc.vector.tensor_tensor(
                    out=dz,
                    in0=xbf[:, w:fsz],
                    in1=xbf[:, 0 : fsz - w],
                    op=AO.subtract,
                )
                sum_relu(dz[:, :], COL_DZP + cidx)
                # first/last plane sums for the signed sums
                sum_ident(xb[:, 0, :], COL_TZF + cidx)
                sum_ident(xb[:, ndh - 1, :], COL_TZL + cidx)

                # boundary pair between the two halves of slice j
                if half == 1:
                    bnd = tinyp.tile([h, w], bf16)
                    sum_max(
                        bnd,
                        xb[:, 0, :],
                        state["prev_xb"][:, ndh - 1, :],
                        COL_BNDP + j,
                    )
                state["prev_xb"] = xb

            # h-diff (gy) via bidiagonal matmul into PSUM; two half-chunk
            # tiles so PE fills one while ScalarE drains the other, and each
            # drain is deferred past the next fill.
            if "gy" in skip:
                return
            for sub in range(nsub):
                ps = psp.tile([h, hb, 512], f32)
                for blk in range(hb):
                    g = sub * hb + blk
                    nc.tensor.matmul(
                        ps[:, blk, :],
                        bd_sb,
                        xb[:, g * planes_per_blk : (g + 1) * planes_per_blk, :],
                        start=True,
                        stop=True,
                    )
                if state["pending_gy"] is not None:
                    emit_gy(*state["pending_gy"])
                state["pending_gy"] = (ps, cidx, sub)

        def chunk_body(j, half):
            cidx = j * 2 + half
            d0 = half * ndh

            # 1) load chunk: [h partitions, ndh planes, w] fp32
            x32 = x32p.tile([h, ndh, w], f32)
            nc.sync.dma_start(
                out=x32, in_=seg[j, d0 : d0 + ndh, :, :].rearrange("d h w -> h d w")
            )

            # 2) cast to bf16; fused accum -> crown/root sum for this chunk
            if "conv" in skip:
                return
            xb = xbp.tile([h, ndh, w], bf16)
            nc.scalar.activation(
                out=xb,
                in_=x32,
                func=AF.Copy,
                accum_out=acc[:, COL_X + cidx : COL_X + cidx + 1],
            )
            xbf = xb.rearrange("p a b -> p (a b)")

            # 3) w-diff (gx): one fused op per chunk.  The exact 3D AP
            #    (misaligned by one element) runs at 1x either way, so it
            #    reads the fp32 tile directly: no dependency on the cast,
            #    and full fp32 precision for the gx term.
            if "dx" not in skip:
                dx = dxp.tile([h, ndh, w - 1], bf16)
                sum_max(dx, x32[:, :, 1:], x32[:, :, 0 : w - 1], COL_DXP + cidx)
                # boundary-column sums for the signed sums (fp32)
                sum_ident(x32[:, :, 0:1], COL_TXF + cidx)
                sum_ident(x32[:, :, w - 1 : w], COL_TXL + cidx)

            # 4) defer xb-dependent work by one chunk so VectorE never
            #    waits on this chunk's cast
            if state["pending_c"] is not None:
                stage_c(*state["pending_c"])
            state["pending_c"] = (j, half, cidx, xb, xbf)

        def all_chunks():
            for j in range(jpc):
                for half in range(2):
                    chunk_body(j, half)
            if state["pending_c"] is not None:
                stage_c(*state["pending_c"])
            state["pending_c"] = None
            if state["pending_gy"] is not None:
                emit_gy(*state["pending_gy"])
            state["pending_gy"] = None

        if repeat == 1:
            all_chunks()
        else:
            with tc.For_i(0, repeat, 1):
                all_chunks()
        nc.sync.dma_start(out=out, in_=acc)

    nc.compile()
    return nc


def _get_program():
    key = "full"
    if key not in _PROG_CACHE:
        _PROG_CACHE[key] = _build_program()
    return _PROG_CACHE[key]


def _bidiag_np(h=H):
    """lhsT for the h-diff matmul: out[m,:] = rhs[m+1,:] - rhs[m,:]."""
    import ml_dtypes

    m = np.zeros((h, h), dtype=np.float32)
    for c in range(h - 1):
        m[c + 1, c] = 1.0
        m[c, c] = -1.0
    # last column stays zero -> output row h-1 is 0
    return m.astype(ml_dtypes.bfloat16)


def _combine(partials, b=B, c=C, d=D, h=H, w=W):
    """Host-side finish: per-core [128, 160] fp32 partials -> [3] fp32."""
    nslice = b * c
    jpc = nslice // len(partials)

    crown = np.zeros(nslice, dtype=np.float64)
    root = np.zeros(nslice, dtype=np.float64)
    gx_sum = 0.0
    gy_sum = 0.0
    gz_sum = 0.0
    for k, p in enumerate(partials):
        p = p.astype(np.float64)
        xp = p[:, COL_DXP : COL_DXP + 2 * jpc].sum(axis=0)
        txf = p[:, COL_TXF : COL_TXF + 2 * jpc].sum(axis=0)
        txl = p[:, COL_TXL : COL_TXL + 2 * jpc].sum(axis=0)
        zp = p[:, COL_DZP : COL_DZP + 2 * jpc].sum(axis=0)
        tzf = p[:, COL_TZF : COL_TZF + 2 * jpc].sum(axis=0)
        tzl = p[:, COL_TZL : COL_TZL + 2 * jpc].sum(axis=0)
        bndp = p[:, COL_BNDP : COL_BNDP + jpc].sum(axis=0)

        xs = p[:, COL_X : COL_X + 2 * jpc].sum(axis=0)
        # sum|a-b| = 2*sum(max(a,b)) - sum(a) - sum(b)
        # gx: a = x[..., 1:], b = x[..., :-1]
        gx_sum += (2.0 * xp - (xs - txf) - (xs - txl)).sum()
        # gz: dz = planes[1:] - planes[:-1]; sum(dz) = tzl - tzf
        gz_sum += (2.0 * zp - (tzl - tzf)).sum()
        # boundary pair: a = half1.plane0, b = half0.plane(ndh-1)
        for jj in range(jpc):
            gz_sum += 2.0 * bndp[jj] - tzf[2 * jj + 1] - tzl[2 * jj]
        gy_sum += p[:, COL_DY : COL_DY + 4 * jpc].sum()

        for jj in range(jpc):
            crown[k * jpc + jj] = p[:, COL_X + 2 * jj].sum()
            root[k * jpc + jj] = p[:, COL_X + 2 * jj + 1].sum()

    total = crown + root
    valid = (total > 0) & (root > 0)
    safe_root = np.where(root > 0, root, 1.0)
    ratio_loss = np.where(valid, (crown / safe_root - EXPECTED_RATIO) ** 2, 0.0)
    cr_loss = ratio_loss.sum() / nslice

    nx = nslice * d * h * (w - 1)
    ny = nslice * d * (h - 1) * w
    nz = nslice * (d - 1) * h * w
    tv = gx_sum / nx + gy_sum / ny + gz_sum / nz

    crown_root = cr_loss * CROWN_ROOT_W
    smoothness = tv * SMOOTH_W
    return np.array(
        [crown_root, smoothness, crown_root + smoothness], dtype=np.float32
    )


def kernel(segmentation: np.ndarray) -> np.ndarray:
    global last_exec_time_ns
    from concourse.bass_utils import run_bass_kernel_spmd

    seg = np.ascontiguousarray(np.asarray(segmentation), dtype=np.float32)
    assert seg.shape == (B, C, D, H, W)
    nc = _get_program()

    bd = _bidiag_np()
    shards = seg.reshape(B * C, D, H, W)
    in_maps = [
        {"seg": np.ascontiguousarray(shards[k * JPC : (k + 1) * JPC]), "bidiag": bd}
        for k in range(NCORES)
    ]
    trace = bool(os.environ.get("BASS_TRACE"))
    res = run_bass_kernel_spmd(nc, in_maps, list(range(NCORES)), trace=trace)
    last_exec_time_ns = res.exec_time_ns
    partials = [res.results[k]["partials"] for k in range(NCORES)]
    return _combine(partials)
